# revision 72
# baseline (speedup 1.0000x reference)
"""Trainium2 Bass kernel for nn_MixClassificationBigSNN (LIF SNN classifier).

Strategy (pure data parallel, 8 NeuronCores; batch 4096 -> 512 rows/core,
everything transposed: feature/hidden dim on partitions, batch on free dim):

  - Encoder elimination: the ConstantCurrentLIFEncoder's membrane resets to
    exactly 0 on spike, so each neuron's spike train is exactly periodic with
    period k determined by thresholding cur = 2*fs*x against fp32 breakpoints
    gamma_k (computed host-side by bisection over the bit-exact fp32 recurrence
    - verified to reproduce the jax encoder spike-for-spike). The per-step
    encoder state updates disappear; instead h_k = (cur > gamma_k) is computed
    once per k on DVE, H_k = h_k @ W1e is cached in SBUF, and the layer-1 drive
    y1(t) = s(t)@W1e = sum_{k | t} (H_k - H_{k-1}) is assembled per step from
    a few cached-H adds (divisor calculus, consecutive-divisor cancellation).
  - Matmul precision: PE fp32r mode runs at 1 cycle/row (bf16 speed) and
    rounds operands to 12-bit significand. Splitting fp32 weights into
    truncate-12 + exact-residual planes reproduces fp32 weights EXACTLY with
    2 planes (vs 3 bf16 planes at 1.5x the cost). Spikes are 0/1, exact in
    fp32r. Wout uses 1 plane (readout error ~1e-4 relative, spike-path free).
  - Readout is linear: v_li(24) = sum_t alpha_t * (z3(t) @ Wout.T) with
    alpha_t = 0.9^(24-t) - 0.8^(24-t), accumulated in PSUM with per-step
    prescaled Wout columns.
  - LIF state: vdec = 0.9v + u (STT/DVE), z = (vdec > th) (TS/DVE for L1,
    sign+relu on ACT for L2/L3), mask = (vdec <= th) (TS/DVE, bf16),
    v' = mask * vdec (TT/GpSimd), u' = 0.8u + y (STT/DVE). Combo adds for
    y1 split between GpSimd and DVE. Hidden u is prescaled (u = 0.1*i) with
    0.1*1.2*(5*es) folded into weights.
  - Dead-code horizons: u1 starts at t=1 (period-1 bin), each later stage one
    step later; everything ends at the last step that can affect v_li(24).
"""
import numpy as np

try:
    import concourse.bass as bass  # noqa: F401
except ImportError:
    import sys
    sys.path.insert(0, "/opt/trn_rl_repo")

import concourse.bass as bass
import concourse.mybir as mybir
from concourse.bass_utils import run_bass_kernel_spmd
from concourse.tile import TileContext

f32 = np.float32

N_CORES = 8
B = 4096
F = 1024
H = 256
C = 10
T = 24
TSTEPS = T - 1      # 23: step 24 never affects the output
KMAX = 20           # max encoder period that can fire within the horizon
BC = B // N_CORES   # 512 batch rows per core
KF = F // 128       # 8 k-tiles for layer 1
KH = H // 128       # 2 k-tiles for layers 2/3/out
NEB = 4 * BC        # 2048: encoder tile free dim (4 k-tiles merged)
NHB = KH * BC       # 1024: hidden merged free dim

ALU = mybir.AluOpType
DT = mybir.dt
AF = mybir.ActivationFunctionType

LAST_RESULTS = None


# ---------------------------------------------------------------- host math

def _first_spike_step(c, maxt=KMAX):
    """Bit-exact fp32 replica of the jax encoder recurrence for scalar cur."""
    v = f32(0.0)
    c = f32(c)
    for j in range(1, maxt + 1):
        v = f32(v + f32(f32(0.1) * f32(c - v)))
        if v > f32(1.0):
            return j
    return 0


def _gamma_table():
    """gamma_k = largest fp32 cur that does NOT spike within k steps."""
    gammas = {}
    for k in range(1, KMAX + 1):
        lo = np.float32(0.1)
        hi = np.float32(1e6)
        lo_b, hi_b = lo.view(np.uint32), hi.view(np.uint32)
        while hi_b - lo_b > 1:
            mid_b = np.uint32((int(lo_b) + int(hi_b)) // 2)
            s = _first_spike_step(mid_b.view(np.float32))
            if 0 < s <= k:
                hi_b = mid_b
            else:
                lo_b = mid_b
        gammas[k] = lo_b.view(np.float32)
    return gammas


def _trunc12(w):
    """Truncate fp32 array to 12-bit significand (fp32r grid, toward zero)."""
    bits = np.ascontiguousarray(w, f32).view(np.uint32)
    return (bits & np.uint32(0xFFFFF000)).view(f32).copy()


def _round12(w):
    """Round fp32 array to nearest 12-bit significand value."""
    w64 = np.asarray(w, np.float64)
    with np.errstate(divide="ignore"):
        e = np.floor(np.log2(np.abs(w64) + 1e-300))
    scale = 2.0 ** (e - 11)
    return (np.round(w64 / scale) * scale).astype(f32)


def _fp32r_planes(w):
    """Split fp32 array into 2 fp32r planes summing exactly to w."""
    p1 = _trunc12(w)
    p2 = (w.astype(np.float64) - p1.astype(np.float64)).astype(f32)
    assert np.array_equal(_trunc12(p2), p2), "residual not fp32r-representable"
    rec = p1.astype(np.float64) + p2.astype(np.float64)
    assert np.array_equal(rec.astype(f32), w.astype(f32))
    assert np.abs(rec - w.astype(np.float64)).max() == 0.0
    return p1, p2


def _combo_terms(occ):
    """Per-step signed H-basis terms: s(t) = sum_{k|t} (h_k - h_{k-1}) with
    consecutive-divisor cancellation; empty bin k maps h_k to the largest
    occupied k' <= k (h_0 = 0). Cached tiles are H_j = h_j @ W1e directly
    (one PSUM copy each, no subtract op). Returns (terms[t] = [(j, sign)...]
    positives first, needed js)."""
    kmap = {0: 0}
    for k in range(1, KMAX + 1):
        kmap[k] = k if occ[k] else kmap[k - 1]
    terms = {}
    needed = set()
    for t in range(1, KMAX + 1):
        cnt = {}
        for k in range(1, KMAX + 1):
            if t % k == 0:
                for j, s in ((kmap[k], 1), (kmap[k - 1], -1)):
                    if j > 0:
                        cnt[j] = cnt.get(j, 0) + s
        lst = [(j, s) for j, s in sorted(cnt.items()) if s != 0]
        assert all(s in (1, -1) for _, s in lst)
        lst.sort(key=lambda js: (-js[1], js[0]))
        if lst:
            assert lst[0][1] == 1
        terms[t] = lst
        needed.update(j for j, _ in lst)
    return terms, sorted(needed)


# ---------------------------------------------------------------- program

def _register_const(nc, dtype, value):
    tensor = nc.alloc_sbuf_tensor(f"const-{dtype.name}-{value}", [128, 1], dtype)
    nc.gpsimd.memset(tensor.ap(), value)
    nc.const_aps.aps[(dtype, value)] = tensor.ap()


def _build_program(gammas, terms, h_js, start, tmax=TSTEPS, dbg=False):
    """start: dict with first active step per stage (u1, z1, u2, z2, u3, z3)."""
    nc = bass.Bass("TRN2", target_bir_lowering=False, debug=False)
    _register_const(nc, DT.float32, -0.33)
    # h-compares mostly on ACT; ACT sign needs a const bias AP per gamma
    act_h = {j for i, j in enumerate(sorted(h_js)) if i % 4 != 0}
    for j in act_h:
        _register_const(nc, DT.float32, -float(gammas[j]))
    nc.all_engine_barrier()

    U1_T, Z1_T = start["u1"], start["z1"]
    U2_T, Z2_T = start["u2"], start["z2"]
    U3_T, Z3_T = start["u3"], start["z3"]
    NT_OUT = TSTEPS - Z3_T + 1       # readout steps

    cur_dram = nc.dram_tensor("cur", [F, BC], DT.float32, kind="ExternalInput").ap()
    w1_dram = nc.dram_tensor("w1", [2, KF, 128, H], DT.float32r, kind="ExternalInput").ap()
    w2_dram = nc.dram_tensor("w2", [2, KH, 128, H], DT.float32r, kind="ExternalInput").ap()
    w3_dram = nc.dram_tensor("w3", [2, KH, 128, H], DT.float32r, kind="ExternalInput").ap()
    wo_dram = nc.dram_tensor("wo", [KH, 128, NT_OUT * C], DT.float32r, kind="ExternalInput").ap()
    out_dram = nc.dram_tensor("out", [C, BC], DT.float32, kind="ExternalOutput").ap()

    # D_j production step: just-in-time (first use t=j; P-build needs it at
    # t-1), spreading the layer-1 matmuls evenly across the run instead of
    # saturating the PE in the first 10 steps.
    h_sched = {}
    for j in sorted(h_js):
        h_sched.setdefault(max(1, j - 2), []).append(j)

    with TileContext(nc) as tc:
        with (
            tc.tile_pool(name="const", bufs=1) as constp,
            tc.tile_pool(name="hcache", bufs=1) as hcp,
            tc.tile_pool(name="state", bufs=1) as statep,
            tc.tile_pool(name="vdecp", bufs=1) as vdecp,
            tc.tile_pool(name="sgp", bufs=1) as sgp,
            tc.tile_pool(name="zp", bufs=1) as zp,
            tc.tile_pool(name="hp", bufs=1) as hp,
            tc.tile_pool(name="pp", bufs=2) as pp,
            tc.tile_pool(name="hyp", bufs=1, space="PSUM") as hyp,
            tc.tile_pool(name="y2p", bufs=1, space="PSUM") as y2p,
            tc.tile_pool(name="y3p", bufs=1, space="PSUM") as y3p,
            tc.tile_pool(name="youtp", bufs=1, space="PSUM") as youtp,
        ):
            # ---- load constants ----
            cur_sb = []
            for g in range(2):
                t_ = constp.tile([128, NEB], DT.float32, tag=f"cur{g}")
                for j in range(4):
                    kk = g * 4 + j
                    nc.sync.dma_start(
                        t_[:, j * BC:(j + 1) * BC],
                        cur_dram[kk * 128:(kk + 1) * 128, :],
                    )
                cur_sb.append(t_)

            def load_w(dram, ktiles, cols, name, planes=2):
                tiles = []
                for s in range(planes):
                    row = []
                    for k in range(ktiles):
                        t_ = constp.tile([128, cols], DT.float32r, tag=f"{name}_{s}_{k}")
                        nc.sync.dma_start(t_[:], dram[s, k] if planes > 1 else dram[k])
                        row.append(t_)
                    tiles.append(row)
                return tiles

            # w1 + cur gate the first D-matmuls; w2/w3/wo aren't needed
            # until step Z1_T, so load them last.
            w1_sb = load_w(w1_dram, KF, H, "w1")
            w2_sb = load_w(w2_dram, KH, H, "w2")
            w3_sb = load_w(w3_dram, KH, H, "w3")
            wo_sb = load_w(wo_dram, KH, NT_OUT * C, "wo", planes=1)
            del w1_dram, w2_dram, w3_dram, wo_dram

            # ---- state init (u updated in place) ----
            v_t = {}
            u_t = {}
            for l in (1, 2, 3):
                vt_ = statep.tile([128, NHB], DT.float32, tag=f"v{l}")
                nc.gpsimd.memset(vt_[:], 0.0)
                v_t[l] = vt_
                ut_ = statep.tile([128, NHB], DT.float32, tag=f"u{l}")
                nc.gpsimd.memset(ut_[:], 0.0)
                u_t[l] = ut_
            v = {l: (lambda l=l: v_t[l][:]) for l in (1, 2, 3)}
            u = {l: (lambda l=l: u_t[l][:]) for l in (1, 2, 3)}

            Dc = {}  # cached H_j tiles (SBUF, fp32)
            Pt = {}  # prebuilt per-step combo partials: (tile, polarity)
            yout = youtp.tile([C, BC], DT.float32, tag="yout")

            STT = nc.vector.scalar_tensor_tensor
            TS = nc.vector.tensor_scalar
            ACT = nc.scalar.activation

            n_mm_out_total = NT_OUT * KH

            def produce_H(j):
                """h_j = (cur > gamma_j) -> H_j = h_j @ W1e (2 fp32r planes).

                H_1's bin is near-empty (a handful of elements globally), so a
                single weight plane is enough there (error ~1e-7 absolute)."""
                hg = []
                for g in range(2):
                    ht = hp.tile([128, NEB], DT.float32r, tag=f"hg{g}")
                    if j in act_h:
                        sgw = sgp.tile([128, NEB], DT.bfloat16, tag="sgw")
                        ACT(sgw[:], cur_sb[g][:], AF.Sign, bias=-float(gammas[j]))
                        ACT(ht[:], sgw[:], AF.Relu)
                    else:
                        TS(ht[:], cur_sb[g][:], float(gammas[j]), None, ALU.is_gt)
                    hg.append(ht)
                planes = 1 if j == 1 else 2
                y = hyp.tile([128, NHB], DT.float32, tag="hy")
                for m in range(KH):
                    idx = 0
                    for s in range(planes):
                        for kt in range(KF):
                            nc.tensor.matmul(
                                y[:, m * BC:(m + 1) * BC],
                                w1_sb[s][kt][:, m * 128:(m + 1) * 128],
                                hg[kt // 4][:, (kt % 4) * BC:(kt % 4 + 1) * BC],
                                start=(idx == 0),
                                stop=(idx == planes * KF - 1),
                            )
                            idx += 1
                # cache H_j directly (signed H-basis: no subtract needed)
                Dt = hcp.tile([128, NHB], DT.float32, tag=f"D{j}")
                nc.scalar.copy(Dt[:], y[:])      # ACT copies PSUM -> SBUF
                Dc[j] = Dt

            # v is stored NEGATED (nv = -v'): vdec = -0.9*nv + u into a fresh
            # vdec tile; reset makes nv' = (z - 1) * vdec. z via ACT sign+relu.
            def spikes(layer):
                vdec = vdecp.tile([128, NHB], DT.float32, tag=f"vd{layer}")
                STT(vdec[:], v[layer](), -0.9, u[layer](), ALU.mult, ALU.add)
                z = zp.tile([128, NHB], DT.float32r, tag=f"z{layer}")
                if layer == 1:
                    # single DVE op: no ACT queue hop on the z1 -> mm2 chain
                    TS(z[:], vdec[:], 0.33, None, ALU.is_gt)
                else:
                    sg = sgp.tile([128, NHB], DT.bfloat16, tag="sg")
                    ACT(sg[:], vdec[:], AF.Sign, bias=-0.33)
                    ACT(z[:], sg[:], AF.Relu)
                return vdec, z

            def reset_v(layer, vdec, z):
                STT(v[layer](), z[:], 1.0, vdec[:], ALU.subtract, ALU.mult)

            def update_u(uw, yw):
                """u' = 0.8*u + y, in place."""
                STT(uw, uw, 0.8, yw, ALU.mult, ALU.add)

            def build_partial(t):
                """Prebuild P_t = |sum(terms[t][1:])| on GpSimd (off the
                critical chain, issued a step ahead); polarity tracks the sign
                flip when the remaining terms are all negative."""
                lst = terms[t]
                if len(lst) < 3:
                    return
                rest = lst[1:]
                pol = rest[0][1]   # +1: mixed starting positive; -1: all negative
                p = pp.tile([128, NHB], DT.float32, tag="P")
                op0 = ALU.add if rest[1][1] == rest[0][1] else ALU.subtract
                nc.gpsimd.tensor_tensor(p[:], Dc[rest[0][0]][:], Dc[rest[1][0]][:], op0)
                for jj, sg in rest[2:]:
                    op = ALU.add if sg == pol else ALU.subtract
                    nc.gpsimd.tensor_tensor(p[:], p[:], Dc[jj][:], op)
                Pt[t] = (p, pol)

            def update_u1(t):
                """u1' = 0.8*u1 + H_first (+- P_t or +- H_second), in place."""
                lst = terms[t]
                if not lst:
                    TS(u[1](), u[1](), 0.8, None, ALU.mult)
                    return
                update_u(u[1](), Dc[lst[0][0]][:])
                if len(lst) == 2:
                    op = ALU.add if lst[1][1] > 0 else ALU.subtract
                    nc.vector.tensor_tensor(u[1](), u[1](), Dc[lst[1][0]][:], op)
                elif len(lst) >= 3:
                    p, pol = Pt.pop(t)
                    op = ALU.add if pol > 0 else ALU.subtract
                    nc.vector.tensor_tensor(u[1](), u[1](), p[:], op)

            def matmul_hidden(w_sb, z, psum_pool, tag):
                y = psum_pool.tile([128, NHB], DT.float32, tag=tag)
                for m in range(KH):
                    idx = 0
                    for s in range(2):
                        for kt in range(KH):
                            nc.tensor.matmul(
                                y[:, m * BC:(m + 1) * BC],
                                w_sb[s][kt][:, m * 128:(m + 1) * 128],
                                z[:, kt * BC:(kt + 1) * BC],
                                start=(idx == 0),
                                stop=(idx == 2 * KH - 1),
                            )
                            idx += 1
                return y

            dbg_drams = {}
            if dbg:
                for nm in ("du1", "du2", "du3", "dv1", "dv2", "dv3"):
                    dbg_drams[nm] = nc.dram_tensor(
                        nm, [128, NHB], DT.float32, kind="ExternalOutput").ap()
                for j in h_js:
                    dbg_drams[f"dH{j}"] = nc.dram_tensor(
                        f"dH{j}", [128, NHB], DT.float32, kind="ExternalOutput").ap()

            # ---- time loop ----
            for t in range(1, tmax + 1):
                z1a = (Z1_T <= t <= Z1_T + 19)
                z2a = (Z2_T <= t <= Z2_T + 19)
                z3a = (Z3_T <= t <= Z3_T + 19)

                # per-layer decay + spike extraction (pipelines freely)
                vd1 = z1 = vd2 = z2 = vd3 = z3 = None
                if z1a:
                    vd1, z1 = spikes(1)
                if z2a:
                    vd2, z2 = spikes(2)
                if z3a:
                    vd3, z3 = spikes(3)

                # encoder D production (just-in-time)
                for j in h_sched.get(t, []):
                    produce_H(j)

                # layer 1 drive from cached D's
                if U1_T <= t <= KMAX:
                    update_u1(t)
                if U1_T <= t + 1 <= KMAX:
                    build_partial(t + 1)
                # resets issued before the u-updates: DVE executes in order,
                # and the u-STTs stall on matmul PSUM while resets are ready
                # as soon as z arrives (avoids head-of-line blocking).
                if Z1_T <= t <= KMAX:
                    reset_v(1, vd1, z1)
                if z2a and t <= Z2_T + 18:
                    reset_v(2, vd2, z2)
                if z3a and t <= Z3_T + 18:
                    reset_v(3, vd3, z3)

                if z1a:
                    y2 = matmul_hidden(w2_sb, z1[:], y2p, "y2")
                    update_u(u[2](), y2[:])
                if z2a:
                    y3 = matmul_hidden(w3_sb, z2[:], y3p, "y3")
                    update_u(u[3](), y3[:])
                if z3a:
                    # readout accumulation
                    last_out_t = min(Z3_T + 19, tmax)
                    for kt in range(KH):
                        nc.tensor.matmul(
                            yout[:],
                            wo_sb[0][kt][:, (t - Z3_T) * C:(t - Z3_T + 1) * C],
                            z3[:, kt * BC:(kt + 1) * BC],
                            start=(t == Z3_T and kt == 0),
                            stop=(t == last_out_t and kt == KH - 1),
                            skip_group_check=True,
                        )

            out_sb = constp.tile([C, BC], DT.float32, tag="outsb")
            nc.scalar.copy(out_sb[:], yout[:])
            nc.gpsimd.dma_start(out_dram[:], out_sb[:])

            if dbg:
                for l in (1, 2, 3):
                    nc.sync.dma_start(dbg_drams[f"du{l}"], u[l]())
                    nc.sync.dma_start(dbg_drams[f"dv{l}"], v[l]())
                for j in h_js:
                    if j in Dc:
                        nc.sync.dma_start(dbg_drams[f"dH{j}"], Dc[j][:])

    _split_multiwait(nc)
    return nc


def _split_multiwait(nc, cap=1):
    """walrus only honors one inline sync wait on several instruction structs
    (custom STT, pseudo DMA, NOP/drain). Hoist extra waits onto single-wait
    NOPs inserted just before the instruction on the same engine."""
    n_new = 0
    for fn in nc.m.functions:
        for blk in fn.blocks:
            new_list = []
            for ins in blk.instructions:
                si = getattr(ins, "sync_info", None)
                if si is not None and si.on_wait and len(si.on_wait) > cap:
                    waits = list(si.on_wait)
                    extra, keep = waits[:-cap], waits[-cap:]
                    for j, w in enumerate(extra):
                        nop = mybir.InstNoOp(
                            name=f"{ins.name}-presync{j}",
                            sync_info=mybir.SyncInfo(on_wait=[w], on_update=[]),
                        )
                        nop.engine = ins.engine
                        nc.register_instruction(nop)
                        new_list.append(nop)
                        n_new += 1
                    si.on_wait = keep
                new_list.append(ins)
            blk.instructions[:] = new_list
    return n_new


# ---------------------------------------------------------------- entry

def kernel(x, W1, W2, W3, Wout, feature_scalar, encoder_scalar):
    x = np.asarray(x, f32)
    W1 = np.asarray(W1, f32)
    W2 = np.asarray(W2, f32)
    W3 = np.asarray(W3, f32)
    Wout = np.asarray(Wout, f32)
    fs = f32(np.asarray(feature_scalar).reshape(-1)[0])
    es = f32(np.asarray(encoder_scalar).reshape(-1)[0])

    # encoder current, bit-exact to jax: cur = (2.0*fs)*x in fp32
    cur = (f32(2.0) * fs) * x
    curT = np.ascontiguousarray(cur.T)                        # [F, B]

    gammas = _gamma_table()
    # bin occupancy (global): period k iff gamma_k < cur <= gamma_{k-1}
    gl = np.array([gammas[k] for k in range(1, KMAX + 1)], f32)
    occ = {k: bool(((cur > gl[k - 1]) & ((cur <= gl[k - 2]) if k > 1 else True)).any())
           for k in range(1, KMAX + 1)}
    terms, h_js = _combo_terms(occ)

    # stage start steps
    u1_t = next(t for t in range(1, KMAX + 1) if terms[t])
    start = {"u1": u1_t, "z1": u1_t + 1, "u2": u1_t + 1,
             "z2": u1_t + 2, "u3": u1_t + 2, "z3": u1_t + 3}
    NT_OUT = TSTEPS - start["z3"] + 1

    # fold scales into weights (single rounding from f64), split fp32r planes
    W1e = (0.6 * float(es) * W1.astype(np.float64)).astype(f32)  # 0.1*1.2*5*es
    W2e = (0.12 * W2.astype(np.float64)).astype(f32)
    W3e = (0.12 * W3.astype(np.float64)).astype(f32)

    def prep_w(w_eff, ktiles):
        a = np.ascontiguousarray(w_eff.T)                     # [K, M]
        p1, p2 = _fp32r_planes(a)
        sp = np.stack([p1, p2])                               # [2, K, M]
        return np.ascontiguousarray(sp.reshape(2, ktiles, 128, a.shape[1]))

    w1_np = prep_w(W1e, KF)
    w2_np = prep_w(W2e, KH)
    w3_np = prep_w(W3e, KH)

    # readout: alpha_t-scaled Wout, 1 fp32r plane, packed [KH, 128, NT_OUT*C]
    wo = np.zeros((H, NT_OUT * C), np.float64)
    Wd = Wout.astype(np.float64).T                            # [H, C]
    for i, t in enumerate(range(start["z3"], TSTEPS + 1)):
        alpha = 0.9 ** (T - t) - 0.8 ** (T - t)
        wo[:, i * C:(i + 1) * C] = alpha * Wd
    wo_np = np.ascontiguousarray(
        _round12(wo.astype(f32)).reshape(KH, 128, NT_OUT * C))

    import os
    dbg_tmax = int(os.environ.get("SNN_DBG_TMAX", "0"))
    nc = _build_program(gammas, terms, h_js, start,
                        tmax=dbg_tmax or TSTEPS, dbg=bool(dbg_tmax))

    in_maps = []
    for core in range(N_CORES):
        sl = slice(core * BC, (core + 1) * BC)
        in_maps.append({
            "cur": np.ascontiguousarray(curT[:, sl]),
            "w1": w1_np, "w2": w2_np, "w3": w3_np, "wo": wo_np,
        })

    import os
    trace = bool(os.environ.get("SNN_TRACE"))
    res = run_bass_kernel_spmd(nc, in_maps, core_ids=list(range(N_CORES)), trace=trace)
    global LAST_RESULTS
    LAST_RESULTS = res
    if trace and res.exec_time_ns:
        print(f"HW exec time: {res.exec_time_ns} ns")
    outs = [r["out"] for r in res.results]                    # each [C, BC]
    full = np.concatenate([o.T for o in outs], axis=0)        # [B, C]
    return np.ascontiguousarray(full.astype(f32))
